# revision 3
# baseline (speedup 1.0000x reference)
"""Trainium2 Bass kernel for nn_BasicLSTM: (B,T,N,C) shared-weight LSTM -> FC.

Data parallel over 8 cores (8 batches/core = 10960 seqs, padded to 11264).
Per core: 11 pairs of 1024 seqs; each pair = 2 blocks of 512 stacked on
partitions (block0 -> 0:64, block1 -> 64:128).

Key design points:
  - K=128 zero-padded stationary weights => FWL fast-weight-load, LDWEIGHTS
    fully overlaps matmul streaming (216ns/mm pitch instead of 426).
  - block1's rhs lives at partitions 55:128 of the same [128,...] staging
    tile (x at rows 55:64, h at 64:128, zero weights for rows 0:55), so the
    DVE h-write for block1 needs NO partition-shift DMA at all.
  - x is staged T/4 steps at a time into per-group "mega" tiles by a handful
    of large DMAs (SWDGE on gpsimd) instead of 264 small per-step DMAs.
  - tanh_g, tanh_c and the whole DVE chain run jointly over turn-PAIRS
    (FD=1024) to halve ACT/DVE instruction overheads; sigmoid stays
    per-turn so PSUM ifo banks stay double-buffered (PE/ACT overlap).
  - FC tail via K=128 matmuls on a sanitized h-store, one copy + one cast
    DMA per group.
"""

import numpy as np
from contextlib import ExitStack

import concourse.bass as bass
import concourse.mybir as mybir
import concourse.tile as tile
from concourse import bacc
from concourse.bass_utils import run_bass_kernel_spmd

B, T, N, C, H = 64, 12, 1370, 8, 64
NCORES = 8
BPC = B // NCORES
SEQS = BPC * N                # 10960
S = 512
PAIRS = 11                    # ceil to 11*1024 = 11264
SEQS_PAD = PAIRS * 2 * S      # 11264
KX = C + 1                    # 8 x channels + ones
TQ = 3                        # steps per staged quarter
NQ = T // TQ                  # 4
GROUPS = [(0, 6), (6, 5)]   # (first pair, n pairs)

BF16 = mybir.dt.bfloat16
F32 = mybir.dt.float32
NPBF16 = mybir.dt.np(BF16)
AF = mybir.ActivationFunctionType


def build_nc() -> bass.Bass:
    nc = bacc.Bacc("TRN2", target_bir_lowering=False, debug=False)

    # x split by block: *_lo feeds rows 64:73 (block0), *_hi rows 55:64 (block1)
    # layout [KX, NQ, TQ, PAIRS, S]: (pair, col) contiguous => DMA APs stay <= 3 dims
    xlo = nc.declare_dram_parameter("xlo", [KX, NQ, TQ, PAIRS * S], BF16, isOutput=False)
    xhi = nc.declare_dram_parameter("xhi", [KX, NQ, TQ, PAIRS * S], BF16, isOutput=False)
    wlo = nc.declare_dram_parameter("wlo", [128, 4 * H], BF16, isOutput=False)
    whi = nc.declare_dram_parameter("whi", [128, 4 * H], BF16, isOutput=False)
    wfc = nc.declare_dram_parameter("wfc", [128, 2 * C], BF16, isOutput=False)
    y = nc.declare_dram_parameter("y", [C, PAIRS, 2, S], F32, isOutput=True)

    with tile.TileContext(nc) as tc, ExitStack() as ctx:
        const = ctx.enter_context(tc.tile_pool(name="const", bufs=1))
        megap = ctx.enter_context(tc.tile_pool(name="megap", bufs=3))
        sgp = ctx.enter_context(tc.tile_pool(name="sgp", bufs=3))
        thgp = ctx.enter_context(tc.tile_pool(name="thgp", bufs=3))
        thcp = ctx.enter_context(tc.tile_pool(name="thcp", bufs=3))
        igp = ctx.enter_context(tc.tile_pool(name="igp", bufs=3))
        fcpool = ctx.enter_context(tc.tile_pool(name="fcpool", bufs=3))
        cpool = ctx.enter_context(tc.tile_pool(name="cpool", bufs=6))
        ytp = ctx.enter_context(tc.tile_pool(name="ytp", bufs=1))
        pifo = ctx.enter_context(tc.tile_pool(name="pifo", bufs=2, space="PSUM"))
        pgp = ctx.enter_context(tc.tile_pool(name="pgp", bufs=1, space="PSUM"))

        # ---- constants / warmup --------------------------------------
        wlo_sb = const.tile([128, 4 * H], BF16)
        nc.sync.dma_start(out=wlo_sb[:, :], in_=wlo[:, :])
        whi_sb = const.tile([128, 4 * H], BF16)
        nc.sync.dma_start(out=whi_sb[:, :], in_=whi[:, :])
        wfc_sb = const.tile([128, 2 * C], BF16)
        nc.sync.dma_start(out=wfc_sb[:, :], in_=wfc[:, :])

        # h store for the FC tail (memset/ones emitted later, off the ramp)
        hs = const.tile([128, PAIRS, 2, S], BF16)
        hs_init = [False]

        def init_hs():
            if hs_init[0]:
                return
            hs_init[0] = True
            nc.vector.memset(hs[:, :, :, :], 0.0)
            # ones rows for FC bias: row 64 for block0 windows, row 63 for block1
            ones_lo = xlo[C : C + 1, 0, 0, :].rearrange("k (p s) -> k p s", p=PAIRS)
            ones_hi = xhi[C : C + 1, 0, 0, :].rearrange("k (p s) -> k p s", p=PAIRS)
            nc.sync.dma_start(out=hs[64:65, :, 0:1, :], in_=ones_lo)
            nc.sync.dma_start(out=hs[63:64, :, 1:2, :], in_=ones_hi)

        # ACT table warm load + PE warm-up under the DMA shadow
        scratch = const.tile([128, 512], BF16)
        nc.vector.memset(scratch[:, :], 1.0)
        nc.scalar.activation(scratch[0:1, 0:8], scratch[0:1, 0:8], AF.Sigmoid)
        pwarm = pgp.tile([128, 2, S], F32, name="pwarm", tag="pg")
        for wi in range(6):
            nc.tensor.matmul(
                pwarm[64 * (wi % 2) : 64 * (wi % 2) + 64, wi // 3, :],
                scratch[0:73, 0:64], scratch[0:73, :])

        sanitized = [0]  # first 3 mega buffers get a one-time garbage-row memset

        # gate column order in wlo/whi: i, f, o, g  (i,f,o contiguous for sigmoid)
        GC = [0, H, 2 * H, 3 * H]

        def new_quarter(gi, q):
            """Allocate + x-fill the staging tile for (group gi, quarter q)."""
            p0, npair = GROUPS[gi]
            mg = megap.tile([128, TQ, 6, 2, S], BF16, name="mg", tag="mega")
            cseg = slice(p0 * S, (p0 + npair) * S)
            for tq in range(TQ):
                # block0 x+ones at rows 64:73
                nc.gpsimd.dma_start(
                    out=mg[H : H + KX, tq, 0:npair, 0:1, :],
                    in_=xlo[0:KX, q, tq, cseg],
                )
                if q == 0 and tq == 0:
                    # t=0: block1 x also at rows 64:73 (h region unused at t=0)
                    nc.gpsimd.dma_start(
                        out=mg[H : H + KX, 0, 0:npair, 1:2, :],
                        in_=xhi[0:KX, 0, 0, cseg],
                    )
                else:
                    # block1 x+ones at rows 55:64
                    nc.gpsimd.dma_start(
                        out=mg[H - KX : H, tq, 0:npair, 1:2, :],
                        in_=xhi[0:KX, q, tq, cseg],
                    )
            if sanitized[0] < 3:
                sanitized[0] += 1
                nc.gpsimd.memset(mg[0:55, :, :, 1, :], 0.0)
            return mg

        pending_fc = []
        for gi, (p0, npair) in enumerate(GROUPS):
            tps = []
            lp = 0
            while lp < npair:
                tps.append((lp, min(2, npair - lp)))
                lp += 2
            quarters = {0: new_quarter(gi, 0)}
            cs = {}
            for lp0, cnt in tps:
                cs[lp0] = cpool.tile([128, 2, S], BF16, name="c_t", tag="c_t")

            pend = []

            def back_half(item):
                bt, lp0, cnt, c_t, sg, thg = item
                if bt == 0:
                    nc.vector.tensor_mul(c_t[:, 0:cnt, :], sg[:, 0, 0:cnt, :], thg[:, 0:cnt, :])
                else:
                    ig = igp.tile([128, 2, S], BF16, name="ig", tag="ig")
                    fcx = fcpool.tile([128, 2, S], BF16, name="fcx", tag="fcx")
                    nc.vector.tensor_mul(ig[:, 0:cnt, :], sg[:, 0, 0:cnt, :], thg[:, 0:cnt, :])
                    nc.vector.tensor_mul(fcx[:, 0:cnt, :], sg[:, 1, 0:cnt, :], c_t[:, 0:cnt, :])
                    nc.vector.tensor_add(c_t[:, 0:cnt, :], ig[:, 0:cnt, :], fcx[:, 0:cnt, :])

                thc = thcp.tile([128, 2, S], BF16, name="thc", tag="thc")
                nc.scalar.activation(thc[:, 0:cnt, :], c_t[:, 0:cnt, :], AF.Tanh)

                # h = sigmoid(o) * tanh(c); block0 -> rows 0:64, block1 -> 64:128
                if bt == T - 1:
                    dlo = hs[0:H, p0 + lp0 : p0 + lp0 + cnt, 0, :]
                    dhi = hs[H:128, p0 + lp0 : p0 + lp0 + cnt, 1, :]
                else:
                    mgb = quarters[(bt + 1) // TQ]
                    tqb = (bt + 1) % TQ
                    dlo = mgb[0:H, tqb, lp0 : lp0 + cnt, 0, :]
                    dhi = mgb[H:128, tqb, lp0 : lp0 + cnt, 1, :]
                nc.vector.tensor_mul(dlo, sg[0:H, 2, 0:cnt, :], thc[0:H, 0:cnt, :])
                nc.vector.tensor_mul(dhi, sg[H:128, 2, 0:cnt, :], thc[H:128, 0:cnt, :])

            for t in range(T):
                if gi == 0 and t == T - 1:
                    init_hs()
                if t in (1, 3) and pending_fc:
                    pending_fc.pop(0)()
                q = t // TQ
                tq = t % TQ
                if tq == 0 and q + 1 < NQ:
                    quarters[q + 1] = new_quarter(gi, q + 1)
                mg = quarters[q]

                for lp0, cnt in tps:
                    c_t = cs[lp0]
                    sg = sgp.tile([128, 3, 2, S], BF16, name="sg", tag="sg")
                    pg = pgp.tile([128, 2, S], F32, name="pg", tag="pg")
                    thg = thgp.tile([128, 2, S], BF16, name="thg", tag="thg")

                    # per-turn: 8 matmuls + sigmoid; g matmuls feed the shared pg
                    for j in range(cnt):
                        lp = lp0 + j
                        pi = pifo.tile([128, 3, S], F32, name="pifo", tag="pifo")
                        for gidx in range(4):
                            gof = GC[gidx]
                            for b in (0, 1):
                                if t == 0:
                                    lh = wlo_sb[H : H + KX, gof : gof + H]
                                    rh = mg[H : H + KX, 0, lp, b, :]
                                elif b == 0:
                                    # K=73 serial-LDW: keeps the PE queue busy
                                    # so the HAM clock gate stays at full rate
                                    lh = wlo_sb[0 : H + KX, gof : gof + H]
                                    rh = mg[0 : H + KX, tq, lp, 0, :]
                                else:
                                    lh = whi_sb[:, gof : gof + H]
                                    rh = mg[:, tq, lp, 1, :]
                                if gidx < 3:
                                    dst = pi[64 * b : 64 * b + 64, gidx, :]
                                else:
                                    dst = pg[64 * b : 64 * b + 64, j, :]
                                nc.tensor.matmul(dst, lh, rh)
                        nc.scalar.activation(sg[:, 0:3, j, :], pi[:, 0:3, :], AF.Sigmoid)

                    # joint over the turn-pair (FD = cnt*512)
                    nc.scalar.activation(thg[:, 0:cnt, :], pg[:, 0:cnt, :], AF.Tanh)

                    # back-half (DVE chain, tanh_c, h-write) runs one turn-pair
                    # later so tanh_c never head-blocks the next sigma on the
                    # strict-FIFO ACT queue
                    pend.append((t, lp0, cnt, c_t, sg, thg))
                    if len(pend) > 1:
                        back_half(pend.pop(0))

            while pend:
                back_half(pend.pop(0))

            # ---- FC tail for this group: deferred into the next group's
            # ramp (or emitted now for the last group) -------------------
            def make_fc_batch(j0, nb, p0=p0):
                def do_fc():
                    pf1 = pifo.tile([128, 3, S], F32, name="pf1", tag="pifo")
                    pf2 = pifo.tile([128, 3, S], F32, name="pf2", tag="pifo")
                    pgf = pgp.tile([128, 2, S], F32, name="pgf", tag="pg") if nb == 4 else None
                    yt = ytp.tile([C, 8, S], F32, name="yt", tag="yt")
                    for j in range(nb):
                        for b in (0, 1):
                            k = j * 2 + b
                            if k < 3:
                                dst = pf1[0:C, k, :]
                            elif k < 6:
                                dst = pf2[0:C, k - 3, :]
                            else:
                                dst = pgf[0:C, k - 6, :]
                            nc.tensor.matmul(
                                dst, wfc_sb[:, C * b : C * b + C],
                                hs[:, p0 + j0 + j, b, :])
                    nc.vector.tensor_copy(yt[0:C, 0 : min(3, 2 * nb), :], pf1[0:C, 0 : min(3, 2 * nb), :])
                    if 2 * nb > 3:
                        nc.vector.tensor_copy(yt[0:C, 3 : min(6, 2 * nb), :], pf2[0:C, 0 : min(3, 2 * nb - 3), :])
                    if 2 * nb > 6:
                        nc.vector.tensor_copy(yt[0:C, 6 : 2 * nb, :], pgf[0:C, 0 : 2 * nb - 6, :])
                    nc.sync.dma_start(
                        out=y[0:C, p0 + j0 : p0 + j0 + nb, :, :],
                        in_=yt[0:C, 0 : 2 * nb, :])
                return do_fc
            for j0 in range(0, npair, 4):
                pending_fc.append(make_fc_batch(j0, min(4, npair - j0)))
        while pending_fc:
            pending_fc.pop(0)()

    nc.compile()
    return nc


def prep_inputs(x, W_ih, W_hh, b_ih, b_hh, W_fc, b_fc):
    x = np.asarray(x, dtype=np.float32)
    W_ih = np.asarray(W_ih, dtype=np.float32)
    W_hh = np.asarray(W_hh, dtype=np.float32)
    bias = np.asarray(b_ih, dtype=np.float32) + np.asarray(b_hh, dtype=np.float32)
    W_fc = np.asarray(W_fc, dtype=np.float32)
    b_fc = np.asarray(b_fc, dtype=np.float32)

    # gate column order i, f, o, g (pytorch rows: i, f, g, o)
    gsrc = [0, 1, 3, 2]
    wlo = np.zeros((128, 4 * H), dtype=np.float32)
    whi = np.zeros((128, 4 * H), dtype=np.float32)
    for gd in range(4):
        rows = slice(H * gsrc[gd], H * gsrc[gd] + H)
        cols = slice(H * gd, H * gd + H)
        wlo[0:H, cols] = W_hh[rows, :].T
        wlo[H : H + C, cols] = W_ih[rows, :].T
        wlo[H + C, cols] = bias[rows]
        whi[H - KX : H - 1, cols] = W_ih[rows, :].T
        whi[H - 1, cols] = bias[rows]
        whi[H:128, cols] = W_hh[rows, :].T
    wfc = np.zeros((128, 2 * C), dtype=np.float32)
    wfc[0:H, 0:C] = W_fc.T
    wfc[H, 0:C] = b_fc
    wfc[H - 1, C : 2 * C] = b_fc
    wfc[H:128, C : 2 * C] = W_fc.T

    wlo16 = wlo.astype(NPBF16)
    whi16 = whi.astype(NPBF16)
    wfc16 = wfc.astype(NPBF16)
    xpad = np.zeros((C, T, SEQS_PAD), dtype=np.float32)
    in_maps = []
    for k in range(NCORES):
        xc = x[k * BPC : (k + 1) * BPC]                   # (BPC, T, N, C)
        xt = xc.transpose(3, 1, 0, 2).reshape(C, T, SEQS)
        xpad[:, :, :SEQS] = xt
        x6 = xpad.reshape(C, NQ, TQ, PAIRS, 2, S)
        xlo_ = np.ones((KX, NQ, TQ, PAIRS * S), dtype=NPBF16)
        xhi_ = np.ones((KX, NQ, TQ, PAIRS * S), dtype=NPBF16)
        xlo_[0:C] = x6[:, :, :, :, 0, :].reshape(C, NQ, TQ, PAIRS * S).astype(NPBF16)
        xhi_[0:C] = x6[:, :, :, :, 1, :].reshape(C, NQ, TQ, PAIRS * S).astype(NPBF16)
        in_maps.append(
            {"xlo": xlo_, "xhi": xhi_, "wlo": wlo16, "whi": whi16, "wfc": wfc16}
        )
    return in_maps


_CACHE = {}


def _get_nc():
    if "nc" not in _CACHE:
        _CACHE["nc"] = build_nc()
    return _CACHE["nc"]


def kernel(x, W_ih, W_hh, b_ih, b_hh, W_fc, b_fc, **run_kwargs):
    nc = _get_nc()
    in_maps = prep_inputs(x, W_ih, W_hh, b_ih, b_hh, W_fc, b_fc)
    res = run_bass_kernel_spmd(nc, in_maps, list(range(NCORES)), **run_kwargs)
    outs = res.results
    ys = []
    for k in range(NCORES):
        yk = np.asarray(outs[k]["y"], dtype=np.float32).reshape(C, SEQS_PAD)
        ys.append(yk.T[:SEQS].reshape(BPC, N, C))
    yfull = np.concatenate(ys, axis=0)
    if run_kwargs.get("trace"):
        _CACHE["last_result"] = res
    return yfull.astype(np.float32)
